# revision 12
# baseline (speedup 1.0000x reference)
"""SOM (vq_codebook) update kernel for 8 Trainium2 NeuronCores.

Strategy
--------
The reference updates a 4096x4096 SOM sheet (128x128 units of 32x32 pixels):
  1. unit_map[u] = sum over u's 32x32 block of (som - tile(x))^2 / (rv + eps)
  2. BMU = argmin(unit_map)
  3. neighborhood update of som / running_variance around the BMU with
     radius r = radius[bmu]; outside the disc (cd > r) the update is an
     exact no-op.

Phase 1 (the 64 MiB sheet scan) runs on the 8 NeuronCores, row-sharded
(512 pixel rows = 16 unit rows per core). Per-core pipeline in the fast
path (running_variance uniform, so the 1/(rv+eps) scale cannot change the
argmin):

  * som is cast fp32->bf16 *during* the HBM->SBUF DMA (SWDGE cast is
    measured line-rate), so every on-chip pass runs on 16-bit data.
  * DVE: diff = som - tile(x) as bf16 tensor_tensor (2x mode), plus the
    squares for row-tile 0 (load balancing with ACT).
  * ACT: squares for row-tiles 1-3 (bf16 in / bf16 out).
  * PE:  the 32-row partition-group sums as bf16 indicator matmuls
    (1 cycle/col), accumulating all 4 row-tiles into one [16, 4096]
    PSUM region.
  * DVE: per-512-column tensor_reduce folds the 32-column blocks ->
    [16, 128] unit map slice, DMA'd out.

This keeps every engine under the ~25us DMA floor (8 MiB fp32 som read
per core at ~358 GB/s), instead of the old ~34us of fp32 DVE work.

The device unit map is bf16-accurate (abs err ~0.2 vs an argmin margin of
~30 for the reference input); the host re-checks the top-K candidate units
exactly (float64, 32x32 blocks) and picks the true argmin, so the BMU is
exact regardless of the bf16 noise. The argmin and the neighborhood update
only touch a (2*floor(r)+1)^2-unit bounding box (~0.5% of the sheet), so
they run on the host; the rest of the output is a bitwise copy of the
inputs.
"""

import numpy as np

S = 4096
N = 128
IMG = 32
NCLS = 10
NCORES = 8
ROWS = S // NCORES          # 512 pixel rows per core
TILES = ROWS // 128         # 4 row-tiles of [128, 4096]
UR = ROWS // IMG            # 16 unit rows per core
EPS = 1e-8
RV_ALPHA = 0.9
TOPK = 32                   # host-side exact argmin re-check width

_CACHE = {}


# column regions of the unit sheet: [col, width]. Region 0 streams fp32 on
# the HWDGE (sync) queue whose first byte arrives ~1us in, filling the ~4us
# SWDGE pipeline-start window; the rest stream as f32->bf16 cast DMAs, ONE
# DMA per region carrying all four row-tiles t-interleaved (2 MiB apiece),
# which keeps the descriptor-generation (Q7) pipeline ahead of the SDMA
# drain. Each region's PSUM accumulation closes as its data ends, so the
# unit-map folds run mid-kernel; the last two regions taper for a short
# tail.
R_SYNC = (0, 512)
R_CAST = [(512, 1024), (1536, 1024), (2560, 1024), (3584, 256), (3840, 256)]


def build_fast():
    """bf16 fast-path program (rv uniform): unit scores = sum (som-x)^2.

    Inputs (per core):
      som  [512, 4096] f32 : this core's row shard of the SOM sheet
      xr32 [128, 512]  f32 : input image x pre-tiled (periodic mod 32, so
                             any 32-aligned window sees the same pattern)
      xr16 [128, 2048] bf16: same pattern for the bf16 subtracts
    Output:
      um   [16, 128]  f32  : this core's unit rows of the score map
    """
    import ml_dtypes
    import concourse.bacc as bacc
    import concourse.mybir as mybir
    from concourse import tile

    f32 = mybir.dt.float32
    bf16 = mybir.dt.bfloat16

    nc = bacc.Bacc("TRN2", target_bir_lowering=False, debug=False)

    som_d = nc.dram_tensor("som", [ROWS, S], f32, kind="ExternalInput")
    xr32_d = nc.dram_tensor("xr32", [128, 512], f32, kind="ExternalInput")
    xr16_d = nc.dram_tensor("xr16", [128, 2048], bf16, kind="ExternalInput")
    um_d = nc.dram_tensor("um", [UR, N], f32, kind="ExternalOutput")

    # indicator lhsT for the 32-partition group sums: for row-tile t,
    # column block 16t..16t+15 maps partition k to unit row 4t + k//32
    ind = np.zeros((128, 16 * TILES), np.float32)
    for t in range(TILES):
        for k in range(128):
            ind[k, 16 * t + 4 * t + k // IMG] = 1.0
    ind_d = nc.inline_tensor(ind.astype(ml_dtypes.bfloat16), "ind")

    def interleaved(c, w):
        # all four row-tiles' [c, c+w) slice as one [128, 4w] DMA source
        return som_d[:, c : c + w].rearrange("(t p) w -> p t w", p=128)

    with tile.TileContext(nc) as tc:
        with (
            tc.tile_pool(name="som", bufs=1) as som_pool,
            tc.tile_pool(name="small", bufs=1) as small_pool,
            tc.tile_pool(name="diff", bufs=6) as diff_pool,
            tc.tile_pool(name="sq", bufs=6) as sq_pool,
            tc.tile_pool(name="psum", bufs=1, space="PSUM") as psum_pool,
        ):
            xr32_t = small_pool.tile([128, 512], f32)
            nc.sync.dma_start(xr32_t[:], xr32_d[:])
            som_r0 = som_pool.tile([128, 4 * R_SYNC[1]], f32)
            nc.sync.dma_start(som_r0[:], interleaved(*R_SYNC))
            ind_t = small_pool.tile([128, 16 * TILES], bf16)
            nc.sync.dma_start(ind_t[:], ind_d[:])
            xr16_t = small_pool.tile([128, 2048], bf16)
            nc.sync.dma_start(xr16_t[:], xr16_d[:])

            seg = {}
            for c, w in R_CAST:
                seg[c] = som_pool.tile([128, 4 * w], bf16, name=f"seg{c}")
                nc.gpsimd.dma_start(seg[c][:], interleaved(c, w))

            # one [16, 4096] accumulation region: matmuls add the four
            # row-tiles' partition-group sums region by region
            um_ps = psum_pool.tile([UR, S], f32)
            um_sb = small_pool.tile([UR, N], f32)

            def bank_slices(c, w):
                # split [c, c+w) at 512-col PSUM bank boundaries
                out, a = [], c
                while a < c + w:
                    b = min((a // 512 + 1) * 512, c + w)
                    out.append((a, b - a))
                    a = b
                return out

            def region_compute(c, w, src, dve_frac, sub_w):
                # src holds [128, 4w] t-interleaved; subs/squares may span
                # t-blocks (the x pattern is 32-periodic and the blocks are
                # 32-aligned). dve_frac = leading fraction (in t-blocks) of
                # the squares done on DVE instead of ACT (engine balance).
                diff = diff_pool.tile([128, 4 * w], bf16, tag="diff")
                xr_sub = xr32_t if src is som_r0 else xr16_t
                for o in range(0, 4 * w, sub_w):
                    nc.vector.tensor_sub(
                        diff[:, o : o + sub_w],
                        src[:, o : o + sub_w],
                        xr_sub[:, :sub_w],
                    )
                sq = sq_pool.tile([128, 4 * w], bf16, tag="sq")
                dve_cols = dve_frac * w
                if dve_cols:
                    nc.vector.tensor_mul(
                        sq[:, :dve_cols], diff[:, :dve_cols], diff[:, :dve_cols]
                    )
                for o in range(dve_cols, 4 * w, w):
                    nc.scalar.activation(
                        sq[:, o : o + w], diff[:, o : o + w],
                        mybir.ActivationFunctionType.Square,
                    )
                for t in range(TILES):
                    for a, ww in bank_slices(c, w):
                        nc.tensor.matmul(
                            um_ps[:, a : a + ww],
                            ind_t[:, 16 * t : 16 * (t + 1)],
                            sq[:, t * w + (a - c) : t * w + (a - c) + ww],
                            start=(t == 0),
                            stop=(t == TILES - 1),
                        )
                nc.vector.tensor_reduce(
                    um_sb[:, c // IMG : (c + w) // IMG],
                    um_ps[:, c : c + w].rearrange("p (a b) -> p a b", b=IMG),
                    axis=mybir.AxisListType.X,
                    op=mybir.AluOpType.add,
                )

            region_compute(*R_SYNC, som_r0, dve_frac=0, sub_w=512)
            for c, w in R_CAST:
                region_compute(
                    c, w, seg[c],
                    dve_frac=(4 if w <= 256 else 1),
                    sub_w=min(2048, 4 * w),
                )

            nc.sync.dma_start(um_d[:], um_sb[:])

    nc.finalize()
    return nc


def _act_reciprocal(nc, mybir, out_ap, in_ap, bias):
    """out = 1 / (in + bias) on the scalar engine (general path only)."""
    eng = nc.scalar
    imm = lambda v: mybir.ImmediateValue(dtype=mybir.dt.float32, value=float(v))
    return eng.add_instruction(
        mybir.InstActivation(
            name=eng.bass.get_next_instruction_name(),
            func=mybir.ActivationFunctionType.Reciprocal,
            ins=[eng.lower_ap(in_ap), imm(bias), imm(1.0), imm(0.0)],
            outs=[eng.lower_ap(out_ap)],
        )
    )


def build_general():
    """fp32 general-path program (rv arbitrary): exact scaled unit map.

    unit_map = sum (som - x)^2 * recip(rv + eps), per 32x32 block.
    """
    import concourse.bacc as bacc
    import concourse.mybir as mybir
    from concourse import tile

    f32 = mybir.dt.float32
    nc = bacc.Bacc("TRN2", target_bir_lowering=False, debug=False)

    som_d = nc.dram_tensor("som", [ROWS, S], f32, kind="ExternalInput")
    rv_d = nc.dram_tensor("rv", [ROWS, S], f32, kind="ExternalInput")
    xr_d = nc.dram_tensor("xr", [128, S // 2], f32, kind="ExternalInput")
    um_d = nc.dram_tensor("um", [UR, N], f32, kind="ExternalOutput")

    ind = np.zeros((128, UR * TILES), np.float32)
    for t in range(TILES):
        for k in range(128):
            ind[k, UR * t + TILES * t + k // IMG] = 1.0
    ind_d = nc.inline_tensor(ind, "ind")

    HALVES = 2
    HS = S // HALVES

    with tile.TileContext(nc) as tc:
        with (
            tc.tile_pool(name="som", bufs=3) as som_pool,
            tc.tile_pool(name="rv", bufs=3) as rv_pool,
            tc.tile_pool(name="g", bufs=2) as g_pool,
            tc.tile_pool(name="diff", bufs=2) as diff_pool,
            tc.tile_pool(name="sq", bufs=2) as sq_pool,
            tc.tile_pool(name="red", bufs=4) as red_pool,
            tc.tile_pool(name="small", bufs=1) as small_pool,
            tc.tile_pool(name="psum", bufs=1, space="PSUM") as psum_pool,
        ):
            QS = S // 4
            som_tiles = [
                som_pool.tile([128, S], f32, tag="som", name=f"som_t{t}")
                for t in range(TILES)
            ]
            nc.sync.dma_start(som_tiles[0][:, :QS], som_d[:128, :QS])
            xr_t = small_pool.tile([128, S // 2], f32)
            nc.sync.dma_start(xr_t[:, :QS], xr_d[:, :QS])
            nc.sync.dma_start(xr_t[:, QS:], xr_d[:, QS:])
            for q in range(1, 4):
                nc.sync.dma_start(
                    som_tiles[0][:, QS * q : QS * (q + 1)],
                    som_d[:128, QS * q : QS * (q + 1)],
                )
            ind_t = small_pool.tile([128, UR * TILES], f32)
            nc.sync.dma_start(ind_t[:], ind_d[:])
            rv_tiles = []
            for t in range(1, TILES):
                nc.sync.dma_start(
                    som_tiles[t][:], som_d[128 * t : 128 * (t + 1), :]
                )
            for t in range(TILES):
                rv_t = rv_pool.tile([128, S], f32)
                nc.sync.dma_start(rv_t[:], rv_d[128 * t : 128 * (t + 1), :])
                rv_tiles.append(rv_t)

            um_ps = psum_pool.tile([UR, TILES * N], f32)

            chunks = [(0, QS * q, QS) for q in range(4)]
            chunks += [(t, HS * c, HS) for t in range(1, TILES - 1) for c in range(HALVES)]
            chunks += [(TILES - 1, QS * q, QS) for q in range(4)]
            for t, col, w in chunks:
                som_h = som_tiles[t][:, col : col + w]

                diff_h = diff_pool.tile([128, HS], f32, tag="diff")
                nc.vector.tensor_sub(diff_h[:, :w], som_h, xr_t[:, :w])
                sq_h = sq_pool.tile([128, HS], f32, tag="sq")
                nc.scalar.activation(
                    sq_h[:, :w], diff_h[:, :w], mybir.ActivationFunctionType.Square
                )
                rv_h = rv_tiles[t][:, col : col + w]
                g_h = g_pool.tile([128, HS], f32, tag="g")
                _act_reciprocal(nc, mybir, g_h[:, :w], rv_h, EPS)
                d2g_h = diff_pool.tile([128, HS], f32, tag="d2g")
                nc.vector.tensor_mul(d2g_h[:, :w], sq_h[:, :w], g_h[:, :w])

                wu = w // IMG
                red_h = red_pool.tile([128, S // 2 // IMG], f32, tag="red")
                nc.vector.tensor_reduce(
                    red_h[:, :wu],
                    d2g_h[:, :w].rearrange("p (a b) -> p a b", b=IMG),
                    axis=mybir.AxisListType.X,
                    op=mybir.AluOpType.add,
                )
                nc.tensor.matmul(
                    um_ps[:, N * t + col // IMG : N * t + (col + w) // IMG],
                    ind_t[:, UR * t : UR * (t + 1)],
                    red_h[:, :wu],
                    start=True,
                    stop=True,
                )

            um_sb = small_pool.tile([UR, N], f32)
            nc.vector.tensor_reduce(
                um_sb[:],
                um_ps[:].rearrange("p (t n) -> p n t", t=TILES),
                axis=mybir.AxisListType.X,
                op=mybir.AluOpType.add,
            )
            nc.sync.dma_start(um_d[:], um_sb[:])

    nc.finalize()
    return nc


def _get_nc(fast):
    key = "fast" if fast else "general"
    if key not in _CACHE:
        _CACHE[key] = build_fast() if fast else build_general()
    return _CACHE[key]


def run_phase1(som, rv, x, **spmd_kwargs):
    """Run phase 1 on the 8 NeuronCores. Returns (unit_map, BassKernelResults);
    the unit_map's top-TOPK units contain the reference argmin."""
    from concourse.bass_utils import run_bass_kernel_spmd

    import ml_dtypes

    rv0 = rv.flat[0]
    fast = bool(rv0 + np.float32(EPS) > 0) and not np.any(rv != rv0)
    nc = _get_nc(fast)
    in_maps = []
    if fast:
        xr32 = np.ascontiguousarray(np.tile(x, (128 // IMG, 512 // IMG)))
        xr16 = np.ascontiguousarray(
            np.tile(x, (128 // IMG, 2048 // IMG)).astype(ml_dtypes.bfloat16)
        )
        for c in range(NCORES):
            in_maps.append(
                {
                    "som": som[c * ROWS : (c + 1) * ROWS],
                    "xr32": xr32,
                    "xr16": xr16,
                }
            )
    else:
        xr = np.ascontiguousarray(np.tile(x, (128 // IMG, (S // 2) // IMG)))
        for c in range(NCORES):
            in_maps.append(
                {
                    "som": som[c * ROWS : (c + 1) * ROWS],
                    "rv": rv[c * ROWS : (c + 1) * ROWS],
                    "xr": xr,
                }
            )
    res = run_bass_kernel_spmd(nc, in_maps, list(range(NCORES)), **spmd_kwargs)
    um = np.concatenate([res.results[c]["um"] for c in range(NCORES)], axis=0)
    return um, res


def device_unit_map(som, rv, x):
    return run_phase1(som, rv, x)[0]


def _exact_argmin(um, som, rv, x):
    """Pick the BMU: the device map ranks units (bf16-accurate); the host
    re-evaluates the TOPK best candidates exactly (float64, full
    (som-x)^2/(rv+eps) block sums) and returns the first-index argmin,
    matching jnp.argmin's row-major first-min tie-break."""
    flat_c = np.argpartition(um.ravel(), TOPK - 1)[:TOPK]
    som64 = som.astype(np.float64)
    rv64 = rv.astype(np.float64)
    x64 = x.astype(np.float64)
    best_flat, best_val = None, None
    for fc in sorted(int(f) for f in flat_c):
        ui, uj = fc // N, fc % N
        blk = som64[ui * IMG : (ui + 1) * IMG, uj * IMG : (uj + 1) * IMG]
        rvb = rv64[ui * IMG : (ui + 1) * IMG, uj * IMG : (uj + 1) * IMG]
        val = (((blk - x64) ** 2) / (rvb + float(np.float32(EPS)))).sum()
        if best_val is None or val < best_val:
            best_flat, best_val = fc, val
    return best_flat


def _phase2_host(som, rv, radius, lrs, x, bi, bj):
    """Neighborhood update on the BMU's bounding box, mirroring the reference
    op-for-op in float32. +,-,*,/,clip are IEEE-exact in both numpy and any
    XLA backend; sqrt/exp/sigmoid/log go through this environment's jax so
    the mask boundary (cd > r at cd == r) matches the reference backend.
    """
    import jax
    import jax.numpy as jnp

    f32 = np.float32
    r = f32(radius[bi, bj])
    lr_b = f32(lrs[bi, bj])
    dm = f32(1.0) / (f32(2.0) * r * r)
    log_t = np.asarray(jnp.log(jnp.float32(f32(EPS) / lr_b)), dtype=f32)
    constant = f32(-log_t) / dm

    hw = int(np.floor(float(r)))
    r0u, r1u = max(0, bi - hw), min(N - 1, bi + hw)
    c0u, c1u = max(0, bj - hw), min(N - 1, bj + hw)
    gi_r = np.arange(r0u, r1u + 1)
    gi_c = np.arange(c0u, c1u + 1)
    cd2 = ((gi_r[:, None] - bi) ** 2 + (gi_c[None, :] - bj) ** 2).astype(f32)
    cd = np.asarray(jnp.sqrt(jnp.asarray(cd2)), dtype=f32)

    mask = np.where(cd > r, f32(0.0), f32(1.0))
    lr_reg = lrs[r0u : r1u + 1, c0u : c1u + 1]
    expterm = np.asarray(jnp.exp(jnp.asarray(-cd * dm)), dtype=f32)
    fm = mask * lr_reg * expterm
    sig = np.asarray(jax.nn.sigmoid(jnp.asarray(cd / constant)), dtype=f32)
    va = f32(RV_ALPHA - 0.5) + sig
    va = np.clip(va * mask + (f32(1.0) - mask), f32(0.0), f32(1.0))

    rs, re = r0u * IMG, (r1u + 1) * IMG
    cs, ce = c0u * IMG, (c1u + 1) * IMG
    fm_big = np.repeat(np.repeat(fm, IMG, 0), IMG, 1)
    va_big = np.repeat(np.repeat(va, IMG, 0), IMG, 1)
    som_r = som[rs:re, cs:ce]
    rv_r = rv[rs:re, cs:ce]
    tiled_r = np.tile(x, (r1u - r0u + 1, c1u - c0u + 1))

    som_new = np.clip(som_r + fm_big * (tiled_r - som_r), f32(0.0), f32(1.0))
    dn = tiled_r - som_new
    rv_new = va_big * rv_r + (f32(1.0) - va_big) * dn * dn
    return (rs, re, cs, ce), som_new, rv_new


def kernel(som, running_variance, radius, learning_rates, class_count, x, y):
    som = np.ascontiguousarray(np.asarray(som, dtype=np.float32))
    rv = np.ascontiguousarray(np.asarray(running_variance, dtype=np.float32))
    radius = np.asarray(radius, dtype=np.float32)
    lrs = np.asarray(learning_rates, dtype=np.float32)
    x32 = np.ascontiguousarray(np.asarray(x, dtype=np.float32))

    um = device_unit_map(som, rv, x32)
    flat = _exact_argmin(um, som, rv, x32)
    bi, bj = flat // N, flat % N

    out = np.empty((2, S, S), np.float32)
    out[0] = som
    out[1] = rv
    (rs, re, cs, ce), som_new, rv_new = _phase2_host(
        som, rv, radius, lrs, x32, bi, bj
    )
    out[0, rs:re, cs:ce] = som_new
    out[1, rs:re, cs:ce] = rv_new
    return out


# revision 16
# speedup vs baseline: 1.0793x; 1.0793x over previous
"""SOM (vq_codebook) update kernel for 8 Trainium2 NeuronCores.

Strategy
--------
The reference updates a 4096x4096 SOM sheet (128x128 units of 32x32 pixels):
  1. unit_map[u] = sum over u's 32x32 block of (som - tile(x))^2 / (rv + eps)
  2. BMU = argmin(unit_map)
  3. neighborhood update of som / running_variance around the BMU with
     radius r = radius[bmu]; outside the disc (cd > r) the update is an
     exact no-op.

Phase 1 (the 64 MiB sheet scan) runs on the 8 NeuronCores, row-sharded
(512 pixel rows = 16 unit rows per core). Per-core pipeline in the fast
path (running_variance uniform, so the 1/(rv+eps) scale cannot change the
argmin):

  * som is cast fp32->bf16 *during* the HBM->SBUF DMA (SWDGE cast is
    measured line-rate), so every on-chip pass runs on 16-bit data.
  * DVE: diff = som - tile(x) as bf16 tensor_tensor (2x mode), plus the
    squares for row-tile 0 (load balancing with ACT).
  * ACT: squares for row-tiles 1-3 (bf16 in / bf16 out).
  * PE:  the 32-row partition-group sums as bf16 indicator matmuls
    (1 cycle/col), accumulating all 4 row-tiles into one [16, 4096]
    PSUM region.
  * DVE: per-512-column tensor_reduce folds the 32-column blocks ->
    [16, 128] unit map slice, DMA'd out.

This keeps every engine under the ~25us DMA floor (8 MiB fp32 som read
per core at ~358 GB/s), instead of the old ~34us of fp32 DVE work.

The device unit map is bf16-accurate (abs err ~0.2 vs an argmin margin of
~30 for the reference input); the host re-checks the top-K candidate units
exactly (float64, 32x32 blocks) and picks the true argmin, so the BMU is
exact regardless of the bf16 noise. The argmin and the neighborhood update
only touch a (2*floor(r)+1)^2-unit bounding box (~0.5% of the sheet), so
they run on the host; the rest of the output is a bitwise copy of the
inputs.
"""

import numpy as np

S = 4096
N = 128
IMG = 32
NCLS = 10
NCORES = 8
ROWS = S // NCORES          # 512 pixel rows per core
TILES = ROWS // 128         # 4 row-tiles of [128, 4096]
UR = ROWS // IMG            # 16 unit rows per core
EPS = 1e-8
RV_ALPHA = 0.9
TOPK = 32                   # host-side exact argmin re-check width

_CACHE = {}


# column regions of the unit sheet: (col, width, n_dma_splits). All som
# data streams as f32->bf16 SWDGE cast DMAs in column-major region order,
# t-interleaved in SBUF ([t0 cols | t1 cols | t2 cols | t3 cols] per
# region) so one DMA can carry several row-tiles. DMA sizing trades two
# hazards: below ~1 MiB the Q7 descriptor-generation pipeline can't stay
# ahead of the SDMA drain (measured ~20% stream stretch), while one DMA
# per region makes compute wait on whole 2 MiB completions. So: the first
# region splits per-tile (0.25 MiB each) purely to start compute early,
# the middle regions stream as 1 MiB tile-pair DMAs, and the small tail
# regions split per-pair so only the last pair's compute chain sits after
# the final byte. Each region's PSUM accumulation closes as its last tile
# lands, spreading the unit-map folds across the kernel instead of piling
# them on the tail.
R_LIST = [
    (0, 512, 4),
    (512, 1024, 2),
    (1536, 1024, 2),
    (2560, 1024, 2),
    (3584, 256, 2),
    (3840, 256, 2),
]


def build_fast():
    """bf16 fast-path program (rv uniform): unit scores = sum (som-x)^2.

    Inputs (per core):
      som  [512, 4096] f32 : this core's row shard of the SOM sheet
      xr16 [128, 2048] bf16: input image x pre-tiled (periodic mod 32, so
                             any 32-aligned window sees the same pattern)
    Output:
      um   [16, 128]  f32  : this core's unit rows of the score map
    """
    import ml_dtypes
    import concourse.bacc as bacc
    import concourse.mybir as mybir
    from concourse import tile

    f32 = mybir.dt.float32
    bf16 = mybir.dt.bfloat16

    nc = bacc.Bacc("TRN2", target_bir_lowering=False, debug=False)

    som_d = nc.dram_tensor("som", [ROWS, S], f32, kind="ExternalInput")
    xr16_d = nc.dram_tensor("xr16", [128, 2048], bf16, kind="ExternalInput")
    um_d = nc.dram_tensor("um", [UR, N], f32, kind="ExternalOutput")

    # indicator lhsT for the 32-partition group sums: for row-tile t,
    # column block 16t..16t+15 maps partition k to unit row 4t + k//32
    ind = np.zeros((128, 16 * TILES), np.float32)
    for t in range(TILES):
        for k in range(128):
            ind[k, 16 * t + 4 * t + k // IMG] = 1.0
    ind_d = nc.inline_tensor(ind.astype(ml_dtypes.bfloat16), "ind")

    def interleaved(c, w, t0, t1):
        # row-tiles [t0, t1)'s [c, c+w) slice as one [128, (t1-t0)*w] source
        return som_d[128 * t0 : 128 * t1, c : c + w].rearrange(
            "(t p) w -> p t w", p=128
        )

    with tile.TileContext(nc) as tc:
        with (
            tc.tile_pool(name="som", bufs=1) as som_pool,
            tc.tile_pool(name="small", bufs=1) as small_pool,
            tc.tile_pool(name="diff", bufs=6) as diff_pool,
            tc.tile_pool(name="sq", bufs=6) as sq_pool,
            tc.tile_pool(name="psum", bufs=1, space="PSUM") as psum_pool,
        ):
            # sync-queue payload is sized to drain inside the ~4us SWDGE
            # pipeline-start window — anything bigger gets starved by the
            # packet-level round-robin once the cast stream starts
            xr16_t = small_pool.tile([128, 2048], bf16)
            nc.sync.dma_start(xr16_t[:], xr16_d[:])
            ind_t = small_pool.tile([128, 16 * TILES], bf16)
            nc.sync.dma_start(ind_t[:], ind_d[:])

            seg = {}
            for c, w, splits in R_LIST:
                seg[c] = som_pool.tile([128, 4 * w], bf16, name=f"seg{c}")
                tt = TILES // splits
                for s in range(splits):
                    nc.gpsimd.dma_start(
                        seg[c][:, s * tt * w : (s + 1) * tt * w],
                        interleaved(c, w, s * tt, (s + 1) * tt),
                    )

            # warm the ACT function table off the critical path (the first
            # Square otherwise eats a ~1.3us table load)
            sq_warm = small_pool.tile([128, 32], bf16)
            nc.scalar.activation(
                sq_warm[:], xr16_t[:, :32], mybir.ActivationFunctionType.Square
            )

            # one [16, 4096] accumulation region: matmuls add the four
            # row-tiles' partition-group sums region by region
            um_ps = psum_pool.tile([UR, S], f32)
            um_sb = small_pool.tile([UR, N], f32)

            for c, w, splits in R_LIST:
                tt = TILES // splits
                sub_w = tt * w
                diff = diff_pool.tile([128, 4 * w], bf16, tag="diff")
                sq = sq_pool.tile([128, 4 * w], bf16, tag="sq")
                for s in range(splits):
                    # subs/squares span this DMA split's t-blocks (the x
                    # pattern is 32-periodic and blocks are 32-aligned)
                    o = s * sub_w
                    nc.vector.tensor_sub(
                        diff[:, o : o + sub_w],
                        seg[c][:, o : o + sub_w],
                        xr16_t[:, :sub_w],
                    )
                    if w <= 256:
                        # tail regions square on DVE (ACT/DVE balance, and
                        # one less cross-engine hop on the kernel tail)
                        nc.vector.tensor_mul(
                            sq[:, o : o + sub_w],
                            diff[:, o : o + sub_w],
                            diff[:, o : o + sub_w],
                        )
                    else:
                        for t in range(s * tt, (s + 1) * tt):
                            nc.scalar.activation(
                                sq[:, t * w : (t + 1) * w],
                                diff[:, t * w : (t + 1) * w],
                                mybir.ActivationFunctionType.Square,
                            )
                    for t in range(s * tt, (s + 1) * tt):
                        for a in range(0, w, 512):
                            ww = min(512, w - a)
                            nc.tensor.matmul(
                                um_ps[:, c + a : c + a + ww],
                                ind_t[:, 16 * t : 16 * (t + 1)],
                                sq[:, t * w + a : t * w + a + ww],
                                start=(t == 0),
                                stop=(t == TILES - 1),
                            )
                nc.vector.tensor_reduce(
                    um_sb[:, c // IMG : (c + w) // IMG],
                    um_ps[:, c : c + w].rearrange("p (a b) -> p a b", b=IMG),
                    axis=mybir.AxisListType.X,
                    op=mybir.AluOpType.add,
                )

            nc.sync.dma_start(um_d[:], um_sb[:])

    nc.finalize()
    return nc


def _act_reciprocal(nc, mybir, out_ap, in_ap, bias):
    """out = 1 / (in + bias) on the scalar engine (general path only)."""
    eng = nc.scalar
    imm = lambda v: mybir.ImmediateValue(dtype=mybir.dt.float32, value=float(v))
    return eng.add_instruction(
        mybir.InstActivation(
            name=eng.bass.get_next_instruction_name(),
            func=mybir.ActivationFunctionType.Reciprocal,
            ins=[eng.lower_ap(in_ap), imm(bias), imm(1.0), imm(0.0)],
            outs=[eng.lower_ap(out_ap)],
        )
    )


def build_general():
    """fp32 general-path program (rv arbitrary): exact scaled unit map.

    unit_map = sum (som - x)^2 * recip(rv + eps), per 32x32 block.
    """
    import concourse.bacc as bacc
    import concourse.mybir as mybir
    from concourse import tile

    f32 = mybir.dt.float32
    nc = bacc.Bacc("TRN2", target_bir_lowering=False, debug=False)

    som_d = nc.dram_tensor("som", [ROWS, S], f32, kind="ExternalInput")
    rv_d = nc.dram_tensor("rv", [ROWS, S], f32, kind="ExternalInput")
    xr_d = nc.dram_tensor("xr", [128, S // 2], f32, kind="ExternalInput")
    um_d = nc.dram_tensor("um", [UR, N], f32, kind="ExternalOutput")

    ind = np.zeros((128, UR * TILES), np.float32)
    for t in range(TILES):
        for k in range(128):
            ind[k, UR * t + TILES * t + k // IMG] = 1.0
    ind_d = nc.inline_tensor(ind, "ind")

    HALVES = 2
    HS = S // HALVES

    with tile.TileContext(nc) as tc:
        with (
            tc.tile_pool(name="som", bufs=3) as som_pool,
            tc.tile_pool(name="rv", bufs=3) as rv_pool,
            tc.tile_pool(name="g", bufs=2) as g_pool,
            tc.tile_pool(name="diff", bufs=2) as diff_pool,
            tc.tile_pool(name="sq", bufs=2) as sq_pool,
            tc.tile_pool(name="red", bufs=4) as red_pool,
            tc.tile_pool(name="small", bufs=1) as small_pool,
            tc.tile_pool(name="psum", bufs=1, space="PSUM") as psum_pool,
        ):
            QS = S // 4
            som_tiles = [
                som_pool.tile([128, S], f32, tag="som", name=f"som_t{t}")
                for t in range(TILES)
            ]
            nc.sync.dma_start(som_tiles[0][:, :QS], som_d[:128, :QS])
            xr_t = small_pool.tile([128, S // 2], f32)
            nc.sync.dma_start(xr_t[:, :QS], xr_d[:, :QS])
            nc.sync.dma_start(xr_t[:, QS:], xr_d[:, QS:])
            for q in range(1, 4):
                nc.sync.dma_start(
                    som_tiles[0][:, QS * q : QS * (q + 1)],
                    som_d[:128, QS * q : QS * (q + 1)],
                )
            ind_t = small_pool.tile([128, UR * TILES], f32)
            nc.sync.dma_start(ind_t[:], ind_d[:])
            rv_tiles = []
            for t in range(1, TILES):
                nc.sync.dma_start(
                    som_tiles[t][:], som_d[128 * t : 128 * (t + 1), :]
                )
            for t in range(TILES):
                rv_t = rv_pool.tile([128, S], f32)
                nc.sync.dma_start(rv_t[:], rv_d[128 * t : 128 * (t + 1), :])
                rv_tiles.append(rv_t)

            um_ps = psum_pool.tile([UR, TILES * N], f32)

            chunks = [(0, QS * q, QS) for q in range(4)]
            chunks += [(t, HS * c, HS) for t in range(1, TILES - 1) for c in range(HALVES)]
            chunks += [(TILES - 1, QS * q, QS) for q in range(4)]
            for t, col, w in chunks:
                som_h = som_tiles[t][:, col : col + w]

                diff_h = diff_pool.tile([128, HS], f32, tag="diff")
                nc.vector.tensor_sub(diff_h[:, :w], som_h, xr_t[:, :w])
                sq_h = sq_pool.tile([128, HS], f32, tag="sq")
                nc.scalar.activation(
                    sq_h[:, :w], diff_h[:, :w], mybir.ActivationFunctionType.Square
                )
                rv_h = rv_tiles[t][:, col : col + w]
                g_h = g_pool.tile([128, HS], f32, tag="g")
                _act_reciprocal(nc, mybir, g_h[:, :w], rv_h, EPS)
                d2g_h = diff_pool.tile([128, HS], f32, tag="d2g")
                nc.vector.tensor_mul(d2g_h[:, :w], sq_h[:, :w], g_h[:, :w])

                wu = w // IMG
                red_h = red_pool.tile([128, S // 2 // IMG], f32, tag="red")
                nc.vector.tensor_reduce(
                    red_h[:, :wu],
                    d2g_h[:, :w].rearrange("p (a b) -> p a b", b=IMG),
                    axis=mybir.AxisListType.X,
                    op=mybir.AluOpType.add,
                )
                nc.tensor.matmul(
                    um_ps[:, N * t + col // IMG : N * t + (col + w) // IMG],
                    ind_t[:, UR * t : UR * (t + 1)],
                    red_h[:, :wu],
                    start=True,
                    stop=True,
                )

            um_sb = small_pool.tile([UR, N], f32)
            nc.vector.tensor_reduce(
                um_sb[:],
                um_ps[:].rearrange("p (t n) -> p n t", t=TILES),
                axis=mybir.AxisListType.X,
                op=mybir.AluOpType.add,
            )
            nc.sync.dma_start(um_d[:], um_sb[:])

    nc.finalize()
    return nc


def _get_nc(fast):
    key = "fast" if fast else "general"
    if key not in _CACHE:
        _CACHE[key] = build_fast() if fast else build_general()
    return _CACHE[key]


def run_phase1(som, rv, x, **spmd_kwargs):
    """Run phase 1 on the 8 NeuronCores. Returns (unit_map, BassKernelResults);
    the unit_map's top-TOPK units contain the reference argmin."""
    from concourse.bass_utils import run_bass_kernel_spmd

    import ml_dtypes

    rv0 = rv.flat[0]
    fast = bool(rv0 + np.float32(EPS) > 0) and not np.any(rv != rv0)
    nc = _get_nc(fast)
    in_maps = []
    if fast:
        xr16 = np.ascontiguousarray(
            np.tile(x, (128 // IMG, 2048 // IMG)).astype(ml_dtypes.bfloat16)
        )
        for c in range(NCORES):
            in_maps.append(
                {"som": som[c * ROWS : (c + 1) * ROWS], "xr16": xr16}
            )
    else:
        xr = np.ascontiguousarray(np.tile(x, (128 // IMG, (S // 2) // IMG)))
        for c in range(NCORES):
            in_maps.append(
                {
                    "som": som[c * ROWS : (c + 1) * ROWS],
                    "rv": rv[c * ROWS : (c + 1) * ROWS],
                    "xr": xr,
                }
            )
    res = run_bass_kernel_spmd(nc, in_maps, list(range(NCORES)), **spmd_kwargs)
    um = np.concatenate([res.results[c]["um"] for c in range(NCORES)], axis=0)
    return um, res


def device_unit_map(som, rv, x):
    return run_phase1(som, rv, x)[0]


def _exact_argmin(um, som, rv, x):
    """Pick the BMU: the device map ranks units (bf16-accurate); the host
    re-evaluates the TOPK best candidates exactly (float64, full
    (som-x)^2/(rv+eps) block sums) and returns the first-index argmin,
    matching jnp.argmin's row-major first-min tie-break."""
    flat_c = np.argpartition(um.ravel(), TOPK - 1)[:TOPK]
    som64 = som.astype(np.float64)
    rv64 = rv.astype(np.float64)
    x64 = x.astype(np.float64)
    best_flat, best_val = None, None
    for fc in sorted(int(f) for f in flat_c):
        ui, uj = fc // N, fc % N
        blk = som64[ui * IMG : (ui + 1) * IMG, uj * IMG : (uj + 1) * IMG]
        rvb = rv64[ui * IMG : (ui + 1) * IMG, uj * IMG : (uj + 1) * IMG]
        val = (((blk - x64) ** 2) / (rvb + float(np.float32(EPS)))).sum()
        if best_val is None or val < best_val:
            best_flat, best_val = fc, val
    return best_flat


def _phase2_host(som, rv, radius, lrs, x, bi, bj):
    """Neighborhood update on the BMU's bounding box, mirroring the reference
    op-for-op in float32. +,-,*,/,clip are IEEE-exact in both numpy and any
    XLA backend; sqrt/exp/sigmoid/log go through this environment's jax so
    the mask boundary (cd > r at cd == r) matches the reference backend.
    """
    import jax
    import jax.numpy as jnp

    f32 = np.float32
    r = f32(radius[bi, bj])
    lr_b = f32(lrs[bi, bj])
    dm = f32(1.0) / (f32(2.0) * r * r)
    log_t = np.asarray(jnp.log(jnp.float32(f32(EPS) / lr_b)), dtype=f32)
    constant = f32(-log_t) / dm

    hw = int(np.floor(float(r)))
    r0u, r1u = max(0, bi - hw), min(N - 1, bi + hw)
    c0u, c1u = max(0, bj - hw), min(N - 1, bj + hw)
    gi_r = np.arange(r0u, r1u + 1)
    gi_c = np.arange(c0u, c1u + 1)
    cd2 = ((gi_r[:, None] - bi) ** 2 + (gi_c[None, :] - bj) ** 2).astype(f32)
    cd = np.asarray(jnp.sqrt(jnp.asarray(cd2)), dtype=f32)

    mask = np.where(cd > r, f32(0.0), f32(1.0))
    lr_reg = lrs[r0u : r1u + 1, c0u : c1u + 1]
    expterm = np.asarray(jnp.exp(jnp.asarray(-cd * dm)), dtype=f32)
    fm = mask * lr_reg * expterm
    sig = np.asarray(jax.nn.sigmoid(jnp.asarray(cd / constant)), dtype=f32)
    va = f32(RV_ALPHA - 0.5) + sig
    va = np.clip(va * mask + (f32(1.0) - mask), f32(0.0), f32(1.0))

    rs, re = r0u * IMG, (r1u + 1) * IMG
    cs, ce = c0u * IMG, (c1u + 1) * IMG
    fm_big = np.repeat(np.repeat(fm, IMG, 0), IMG, 1)
    va_big = np.repeat(np.repeat(va, IMG, 0), IMG, 1)
    som_r = som[rs:re, cs:ce]
    rv_r = rv[rs:re, cs:ce]
    tiled_r = np.tile(x, (r1u - r0u + 1, c1u - c0u + 1))

    som_new = np.clip(som_r + fm_big * (tiled_r - som_r), f32(0.0), f32(1.0))
    dn = tiled_r - som_new
    rv_new = va_big * rv_r + (f32(1.0) - va_big) * dn * dn
    return (rs, re, cs, ce), som_new, rv_new


def kernel(som, running_variance, radius, learning_rates, class_count, x, y):
    som = np.ascontiguousarray(np.asarray(som, dtype=np.float32))
    rv = np.ascontiguousarray(np.asarray(running_variance, dtype=np.float32))
    radius = np.asarray(radius, dtype=np.float32)
    lrs = np.asarray(learning_rates, dtype=np.float32)
    x32 = np.ascontiguousarray(np.asarray(x, dtype=np.float32))

    um = device_unit_map(som, rv, x32)
    flat = _exact_argmin(um, som, rv, x32)
    bi, bj = flat // N, flat % N

    out = np.empty((2, S, S), np.float32)
    out[0] = som
    out[1] = rv
    (rs, re, cs, ce), som_new, rv_new = _phase2_host(
        som, rv, radius, lrs, x32, bi, bj
    )
    out[0, rs:re, cs:ce] = som_new
    out[1, rs:re, cs:ce] = rv_new
    return out


# revision 21
# speedup vs baseline: 1.1245x; 1.0419x over previous
"""SOM (vq_codebook) update kernel for 8 Trainium2 NeuronCores.

Strategy
--------
The reference updates a 4096x4096 SOM sheet (128x128 units of 32x32 pixels):
  1. unit_map[u] = sum over u's 32x32 block of (som - tile(x))^2 / (rv + eps)
  2. BMU = argmin(unit_map)
  3. neighborhood update of som / running_variance around the BMU with
     radius r = radius[bmu]; outside the disc (cd > r) the update is an
     exact no-op.

Phase 1 (the 64 MiB sheet scan) runs on the 8 NeuronCores, row-sharded
(512 pixel rows = 16 unit rows per core). Per-core pipeline in the fast
path (running_variance uniform, so the 1/(rv+eps) scale cannot change the
argmin):

  * som is cast fp32->bf16 *during* the HBM->SBUF DMA (SWDGE cast is
    measured line-rate), so every on-chip pass runs on 16-bit data.
  * DVE: diff = som - tile(x) as bf16 tensor_tensor (2x mode), plus the
    squares for row-tile 0 (load balancing with ACT).
  * ACT: squares for row-tiles 1-3 (bf16 in / bf16 out).
  * PE:  the 32-row partition-group sums as bf16 indicator matmuls
    (1 cycle/col), accumulating all 4 row-tiles into one [16, 4096]
    PSUM region.
  * DVE: per-512-column tensor_reduce folds the 32-column blocks ->
    [16, 128] unit map slice, DMA'd out.

This keeps every engine under the ~25us DMA floor (8 MiB fp32 som read
per core at ~358 GB/s), instead of the old ~34us of fp32 DVE work.

The device unit map is bf16-accurate (abs err ~0.2 vs an argmin margin of
~30 for the reference input); the host re-checks the top-K candidate units
exactly (float64, 32x32 blocks) and picks the true argmin, so the BMU is
exact regardless of the bf16 noise. The argmin and the neighborhood update
only touch a (2*floor(r)+1)^2-unit bounding box (~0.5% of the sheet), so
they run on the host; the rest of the output is a bitwise copy of the
inputs.
"""

import numpy as np

S = 4096
N = 128
IMG = 32
NCLS = 10
NCORES = 8
ROWS = S // NCORES          # 512 pixel rows per core
TILES = ROWS // 128         # 4 row-tiles of [128, 4096]
UR = ROWS // IMG            # 16 unit rows per core
EPS = 1e-8
RV_ALPHA = 0.9
TOPK = 32                   # host-side exact argmin re-check width

_CACHE = {}


# column regions of the unit sheet: (col, width, n_dma_splits). All som
# data streams as f32->bf16 SWDGE cast DMAs in column-major region order,
# t-interleaved in SBUF ([t0 cols | t1 cols | t2 cols | t3 cols] per
# region) so one DMA can carry several row-tiles. DMA sizing trades two
# hazards: below ~1 MiB the Q7 descriptor-generation pipeline can't stay
# ahead of the SDMA drain (measured ~20% stream stretch), while one DMA
# per region makes compute wait on whole 2 MiB completions. So: the first
# region splits per-tile (0.25 MiB each) purely to start compute early,
# the middle regions stream as 1 MiB tile-pair DMAs, and the small tail
# regions split per-pair so only the last pair's compute chain sits after
# the final byte. Each region's PSUM accumulation closes as its last tile
# lands, spreading the unit-map folds across the kernel instead of piling
# them on the tail.
R_LIST = [
    (0, 512, 4),
    (512, 1024, 2),
    (1536, 1024, 2),
    (2560, 1024, 2),
    (3584, 256, 2),
    (3840, 256, 2),
]


def build_fast():
    """bf16 fast-path program (rv uniform): unit scores = sum (som-x)^2.

    Inputs (per core):
      som  [512, 4096] f32 : this core's row shard of the SOM sheet
      xr16 [128, 2048] bf16: input image x pre-tiled (periodic mod 32, so
                             any 32-aligned window sees the same pattern)
    Output:
      um   [16, 128]  f32  : this core's unit rows of the score map
    """
    import ml_dtypes
    import concourse.bacc as bacc
    import concourse.mybir as mybir
    from concourse import tile

    f32 = mybir.dt.float32
    bf16 = mybir.dt.bfloat16
    fp8 = mybir.dt.float8e4

    nc = bacc.Bacc("TRN2", target_bir_lowering=False, debug=False)

    som_d = nc.dram_tensor("som", [ROWS, S], f32, kind="ExternalInput")
    xr16_d = nc.dram_tensor("xr16", [128, 2048], bf16, kind="ExternalInput")
    um_d = nc.dram_tensor("um", [UR, N], f32, kind="ExternalOutput")

    # indicator lhsT for the 32-partition group sums: for row-tile t,
    # column block 16t..16t+15 maps partition k to unit row 4t + k//32.
    # Stored fp8 (0/1 exact) so Double-FP8 matmuls can fold TWO row-tiles
    # per instruction: lhsT [128, 2, 16] + rhs [128, 2, N] contract both
    # k-subtiles at 0.5 cycles/column.
    ind = np.zeros((128, 16 * TILES), np.float32)
    for t in range(TILES):
        for k in range(128):
            ind[k, 16 * t + 4 * t + k // IMG] = 1.0
    ind_d = nc.inline_tensor(ind.astype(ml_dtypes.float8_e4m3), "ind")

    def interleaved(c, w, t0, t1):
        # row-tiles [t0, t1)'s [c, c+w) slice as one [128, (t1-t0)*w] source
        return som_d[128 * t0 : 128 * t1, c : c + w].rearrange(
            "(t p) w -> p t w", p=128
        )

    with tile.TileContext(nc) as tc:
        with (
            tc.tile_pool(name="som", bufs=1) as som_pool,
            tc.tile_pool(name="small", bufs=1) as small_pool,
            tc.tile_pool(name="diff", bufs=6) as diff_pool,
            tc.tile_pool(name="sq", bufs=6) as sq_pool,
            tc.tile_pool(name="psum", bufs=1, space="PSUM") as psum_pool,
        ):
            # sync-queue payload is sized to drain inside the ~4us SWDGE
            # pipeline-start window — anything bigger gets starved by the
            # packet-level round-robin once the cast stream starts
            xr16_t = small_pool.tile([128, 2048], bf16)
            nc.sync.dma_start(xr16_t[:], xr16_d[:])
            ind_t = small_pool.tile([128, 16 * TILES], fp8)
            nc.sync.dma_start(ind_t[:], ind_d[:])

            seg = {}
            for c, w, splits in R_LIST:
                seg[c] = som_pool.tile([128, 4 * w], bf16, name=f"seg{c}")
                tt = TILES // splits
                for s in range(splits):
                    nc.gpsimd.dma_start(
                        seg[c][:, s * tt * w : (s + 1) * tt * w],
                        interleaved(c, w, s * tt, (s + 1) * tt),
                    )

            # warm the ACT function table off the critical path (the first
            # Square otherwise eats a ~1.3us table load)
            sq_warm = small_pool.tile([128, 32], fp8)
            nc.scalar.activation(
                sq_warm[:], xr16_t[:, :32], mybir.ActivationFunctionType.Square
            )

            # one [16, 4096] accumulation region: matmuls add the four
            # row-tiles' partition-group sums region by region
            um_ps = psum_pool.tile([UR, S], f32)
            um_sb = small_pool.tile([UR, N], f32)

            for c, w, splits in R_LIST:
                tt = TILES // splits
                sub_w = tt * w
                diff = diff_pool.tile([128, 4 * w], bf16, tag="diff")
                sq = sq_pool.tile([128, 4 * w], fp8, tag="sq")
                for s in range(splits):
                    # subs/squares span this DMA split's t-blocks (the x
                    # pattern is 32-periodic and blocks are 32-aligned)
                    o = s * sub_w
                    nc.vector.tensor_sub(
                        diff[:, o : o + sub_w],
                        seg[c][:, o : o + sub_w],
                        xr16_t[:, :sub_w],
                    )
                    if w <= 256 and s == splits - 1:
                        # the stream-final split squares on DVE: one less
                        # cross-engine hop on the kernel tail
                        nc.vector.tensor_mul(
                            sq[:, o : o + sub_w],
                            diff[:, o : o + sub_w],
                            diff[:, o : o + sub_w],
                        )
                    else:
                        for oo in range(o, o + sub_w, 2 * w):
                            nc.scalar.activation(
                                sq[:, oo : oo + min(2 * w, sub_w)],
                                diff[:, oo : oo + min(2 * w, sub_w)],
                                mybir.ActivationFunctionType.Square,
                            )
                # Double-FP8 matmuls: pair q folds row-tiles (2q, 2q+1) in
                # one instruction — rhs [128, 2, ww] spans both t-blocks
                for q in range(2):
                    pair = sq[:, 2 * q * w : 2 * (q + 1) * w].rearrange(
                        "p (i n) -> p i n", i=2
                    )
                    lhsT = ind_t[:, 32 * q : 32 * (q + 1)].rearrange(
                        "p (i m) -> p i m", i=2
                    )
                    for a in range(0, w, 512):
                        ww = min(512, w - a)
                        nc.tensor.matmul(
                            um_ps[:, c + a : c + a + ww],
                            lhsT,
                            pair[:, :, a : a + ww],
                            start=(q == 0),
                            stop=(q == 1),
                            perf_mode=mybir.MatmulPerfMode.DoubleRow,
                        )
                nc.vector.tensor_reduce(
                    um_sb[:, c // IMG : (c + w) // IMG],
                    um_ps[:, c : c + w].rearrange("p (a b) -> p a b", b=IMG),
                    axis=mybir.AxisListType.X,
                    op=mybir.AluOpType.add,
                )

            nc.sync.dma_start(um_d[:], um_sb[:])

    nc.finalize()
    return nc


def _act_reciprocal(nc, mybir, out_ap, in_ap, bias):
    """out = 1 / (in + bias) on the scalar engine (general path only)."""
    eng = nc.scalar
    imm = lambda v: mybir.ImmediateValue(dtype=mybir.dt.float32, value=float(v))
    return eng.add_instruction(
        mybir.InstActivation(
            name=eng.bass.get_next_instruction_name(),
            func=mybir.ActivationFunctionType.Reciprocal,
            ins=[eng.lower_ap(in_ap), imm(bias), imm(1.0), imm(0.0)],
            outs=[eng.lower_ap(out_ap)],
        )
    )


def build_general():
    """fp32 general-path program (rv arbitrary): exact scaled unit map.

    unit_map = sum (som - x)^2 * recip(rv + eps), per 32x32 block.
    """
    import concourse.bacc as bacc
    import concourse.mybir as mybir
    from concourse import tile

    f32 = mybir.dt.float32
    nc = bacc.Bacc("TRN2", target_bir_lowering=False, debug=False)

    som_d = nc.dram_tensor("som", [ROWS, S], f32, kind="ExternalInput")
    rv_d = nc.dram_tensor("rv", [ROWS, S], f32, kind="ExternalInput")
    xr_d = nc.dram_tensor("xr", [128, S // 2], f32, kind="ExternalInput")
    um_d = nc.dram_tensor("um", [UR, N], f32, kind="ExternalOutput")

    ind = np.zeros((128, UR * TILES), np.float32)
    for t in range(TILES):
        for k in range(128):
            ind[k, UR * t + TILES * t + k // IMG] = 1.0
    ind_d = nc.inline_tensor(ind, "ind")

    HALVES = 2
    HS = S // HALVES

    with tile.TileContext(nc) as tc:
        with (
            tc.tile_pool(name="som", bufs=3) as som_pool,
            tc.tile_pool(name="rv", bufs=3) as rv_pool,
            tc.tile_pool(name="g", bufs=2) as g_pool,
            tc.tile_pool(name="diff", bufs=2) as diff_pool,
            tc.tile_pool(name="sq", bufs=2) as sq_pool,
            tc.tile_pool(name="red", bufs=4) as red_pool,
            tc.tile_pool(name="small", bufs=1) as small_pool,
            tc.tile_pool(name="psum", bufs=1, space="PSUM") as psum_pool,
        ):
            QS = S // 4
            som_tiles = [
                som_pool.tile([128, S], f32, tag="som", name=f"som_t{t}")
                for t in range(TILES)
            ]
            nc.sync.dma_start(som_tiles[0][:, :QS], som_d[:128, :QS])
            xr_t = small_pool.tile([128, S // 2], f32)
            nc.sync.dma_start(xr_t[:, :QS], xr_d[:, :QS])
            nc.sync.dma_start(xr_t[:, QS:], xr_d[:, QS:])
            for q in range(1, 4):
                nc.sync.dma_start(
                    som_tiles[0][:, QS * q : QS * (q + 1)],
                    som_d[:128, QS * q : QS * (q + 1)],
                )
            ind_t = small_pool.tile([128, UR * TILES], f32)
            nc.sync.dma_start(ind_t[:], ind_d[:])
            rv_tiles = []
            for t in range(1, TILES):
                nc.sync.dma_start(
                    som_tiles[t][:], som_d[128 * t : 128 * (t + 1), :]
                )
            for t in range(TILES):
                rv_t = rv_pool.tile([128, S], f32)
                nc.sync.dma_start(rv_t[:], rv_d[128 * t : 128 * (t + 1), :])
                rv_tiles.append(rv_t)

            um_ps = psum_pool.tile([UR, TILES * N], f32)

            chunks = [(0, QS * q, QS) for q in range(4)]
            chunks += [(t, HS * c, HS) for t in range(1, TILES - 1) for c in range(HALVES)]
            chunks += [(TILES - 1, QS * q, QS) for q in range(4)]
            for t, col, w in chunks:
                som_h = som_tiles[t][:, col : col + w]

                diff_h = diff_pool.tile([128, HS], f32, tag="diff")
                nc.vector.tensor_sub(diff_h[:, :w], som_h, xr_t[:, :w])
                sq_h = sq_pool.tile([128, HS], f32, tag="sq")
                nc.scalar.activation(
                    sq_h[:, :w], diff_h[:, :w], mybir.ActivationFunctionType.Square
                )
                rv_h = rv_tiles[t][:, col : col + w]
                g_h = g_pool.tile([128, HS], f32, tag="g")
                _act_reciprocal(nc, mybir, g_h[:, :w], rv_h, EPS)
                d2g_h = diff_pool.tile([128, HS], f32, tag="d2g")
                nc.vector.tensor_mul(d2g_h[:, :w], sq_h[:, :w], g_h[:, :w])

                wu = w // IMG
                red_h = red_pool.tile([128, S // 2 // IMG], f32, tag="red")
                nc.vector.tensor_reduce(
                    red_h[:, :wu],
                    d2g_h[:, :w].rearrange("p (a b) -> p a b", b=IMG),
                    axis=mybir.AxisListType.X,
                    op=mybir.AluOpType.add,
                )
                nc.tensor.matmul(
                    um_ps[:, N * t + col // IMG : N * t + (col + w) // IMG],
                    ind_t[:, UR * t : UR * (t + 1)],
                    red_h[:, :wu],
                    start=True,
                    stop=True,
                )

            um_sb = small_pool.tile([UR, N], f32)
            nc.vector.tensor_reduce(
                um_sb[:],
                um_ps[:].rearrange("p (t n) -> p n t", t=TILES),
                axis=mybir.AxisListType.X,
                op=mybir.AluOpType.add,
            )
            nc.sync.dma_start(um_d[:], um_sb[:])

    nc.finalize()
    return nc


def _get_nc(fast):
    key = "fast" if fast else "general"
    if key not in _CACHE:
        _CACHE[key] = build_fast() if fast else build_general()
    return _CACHE[key]


def run_phase1(som, rv, x, **spmd_kwargs):
    """Run phase 1 on the 8 NeuronCores. Returns (unit_map, BassKernelResults);
    the unit_map's top-TOPK units contain the reference argmin."""
    from concourse.bass_utils import run_bass_kernel_spmd

    import ml_dtypes

    rv0 = rv.flat[0]
    fast = bool(rv0 + np.float32(EPS) > 0) and not np.any(rv != rv0)
    nc = _get_nc(fast)
    in_maps = []
    if fast:
        xr16 = np.ascontiguousarray(
            np.tile(x, (128 // IMG, 2048 // IMG)).astype(ml_dtypes.bfloat16)
        )
        for c in range(NCORES):
            in_maps.append(
                {"som": som[c * ROWS : (c + 1) * ROWS], "xr16": xr16}
            )
    else:
        xr = np.ascontiguousarray(np.tile(x, (128 // IMG, (S // 2) // IMG)))
        for c in range(NCORES):
            in_maps.append(
                {
                    "som": som[c * ROWS : (c + 1) * ROWS],
                    "rv": rv[c * ROWS : (c + 1) * ROWS],
                    "xr": xr,
                }
            )
    res = run_bass_kernel_spmd(nc, in_maps, list(range(NCORES)), **spmd_kwargs)
    um = np.concatenate([res.results[c]["um"] for c in range(NCORES)], axis=0)
    return um, res


def device_unit_map(som, rv, x):
    return run_phase1(som, rv, x)[0]


def _exact_argmin(um, som, rv, x):
    """Pick the BMU: the device map ranks units (bf16-accurate); the host
    re-evaluates the TOPK best candidates exactly (float64, full
    (som-x)^2/(rv+eps) block sums) and returns the first-index argmin,
    matching jnp.argmin's row-major first-min tie-break."""
    flat_c = np.argpartition(um.ravel(), TOPK - 1)[:TOPK]
    som64 = som.astype(np.float64)
    rv64 = rv.astype(np.float64)
    x64 = x.astype(np.float64)
    best_flat, best_val = None, None
    for fc in sorted(int(f) for f in flat_c):
        ui, uj = fc // N, fc % N
        blk = som64[ui * IMG : (ui + 1) * IMG, uj * IMG : (uj + 1) * IMG]
        rvb = rv64[ui * IMG : (ui + 1) * IMG, uj * IMG : (uj + 1) * IMG]
        val = (((blk - x64) ** 2) / (rvb + float(np.float32(EPS)))).sum()
        if best_val is None or val < best_val:
            best_flat, best_val = fc, val
    return best_flat


def _phase2_host(som, rv, radius, lrs, x, bi, bj):
    """Neighborhood update on the BMU's bounding box, mirroring the reference
    op-for-op in float32. +,-,*,/,clip are IEEE-exact in both numpy and any
    XLA backend; sqrt/exp/sigmoid/log go through this environment's jax so
    the mask boundary (cd > r at cd == r) matches the reference backend.
    """
    import jax
    import jax.numpy as jnp

    f32 = np.float32
    r = f32(radius[bi, bj])
    lr_b = f32(lrs[bi, bj])
    dm = f32(1.0) / (f32(2.0) * r * r)
    log_t = np.asarray(jnp.log(jnp.float32(f32(EPS) / lr_b)), dtype=f32)
    constant = f32(-log_t) / dm

    hw = int(np.floor(float(r)))
    r0u, r1u = max(0, bi - hw), min(N - 1, bi + hw)
    c0u, c1u = max(0, bj - hw), min(N - 1, bj + hw)
    gi_r = np.arange(r0u, r1u + 1)
    gi_c = np.arange(c0u, c1u + 1)
    cd2 = ((gi_r[:, None] - bi) ** 2 + (gi_c[None, :] - bj) ** 2).astype(f32)
    cd = np.asarray(jnp.sqrt(jnp.asarray(cd2)), dtype=f32)

    mask = np.where(cd > r, f32(0.0), f32(1.0))
    lr_reg = lrs[r0u : r1u + 1, c0u : c1u + 1]
    expterm = np.asarray(jnp.exp(jnp.asarray(-cd * dm)), dtype=f32)
    fm = mask * lr_reg * expterm
    sig = np.asarray(jax.nn.sigmoid(jnp.asarray(cd / constant)), dtype=f32)
    va = f32(RV_ALPHA - 0.5) + sig
    va = np.clip(va * mask + (f32(1.0) - mask), f32(0.0), f32(1.0))

    rs, re = r0u * IMG, (r1u + 1) * IMG
    cs, ce = c0u * IMG, (c1u + 1) * IMG
    fm_big = np.repeat(np.repeat(fm, IMG, 0), IMG, 1)
    va_big = np.repeat(np.repeat(va, IMG, 0), IMG, 1)
    som_r = som[rs:re, cs:ce]
    rv_r = rv[rs:re, cs:ce]
    tiled_r = np.tile(x, (r1u - r0u + 1, c1u - c0u + 1))

    som_new = np.clip(som_r + fm_big * (tiled_r - som_r), f32(0.0), f32(1.0))
    dn = tiled_r - som_new
    rv_new = va_big * rv_r + (f32(1.0) - va_big) * dn * dn
    return (rs, re, cs, ce), som_new, rv_new


def kernel(som, running_variance, radius, learning_rates, class_count, x, y):
    som = np.ascontiguousarray(np.asarray(som, dtype=np.float32))
    rv = np.ascontiguousarray(np.asarray(running_variance, dtype=np.float32))
    radius = np.asarray(radius, dtype=np.float32)
    lrs = np.asarray(learning_rates, dtype=np.float32)
    x32 = np.ascontiguousarray(np.asarray(x, dtype=np.float32))

    um = device_unit_map(som, rv, x32)
    flat = _exact_argmin(um, som, rv, x32)
    bi, bj = flat // N, flat % N

    out = np.empty((2, S, S), np.float32)
    out[0] = som
    out[1] = rv
    (rs, re, cs, ce), som_new, rv_new = _phase2_host(
        som, rv, radius, lrs, x32, bi, bj
    )
    out[0, rs:re, cs:ce] = som_new
    out[1, rs:re, cs:ce] = rv_new
    return out
